# revision 26
# baseline (speedup 1.0000x reference)
"""Multi-head attention (B=2, S=4096, D=512, H=8) on 8 trn2 NeuronCores.

Sharding: head-parallel. Core i computes head i for BOTH batches (work per
head is proportional to that batch's valid_len, so pairing each head with
both batches balances the skewed valid_lens across cores). Each core
applies its row-slice of Wo on device and returns a full-shape
UNNORMALIZED partial plus per-query softmax denominators; the host
divides by the denominator and sums the 8 partials (the tensor-parallel
all-reduce, done in the gather step).

Device dataflow per core (matmuls in fp16; PSUM accumulates fp32).
Queries are processed in PAIRS of 512-wide blocks: block 2p lives on SBUF
partitions 0:64 (its 64 head dims), block 2p+1 on partitions 64:128.
K^T is replicated on both partition halves. This lets every stage use
PE-array tiling (tile_position) so two matmuls run concurrently:

  - scores, per (q-block-pair, k-chunk):
        ssc[k,q]  blk0 = kT_lo.T @ qT_lo     (row-tile (0,0))
        ssc[k,q]  blk1 = kT_hi.T @ qT_hi     (row-tile (64,0))   } concurrent
  - exp on ACT in 2-chunk batches (N=2048) to amortize the ~350cy
    per-instruction overhead; boundary chunk masked multiplicatively
    afterwards on DVE (per-partition 0/1 scalar).
  - outU accumulation, per k-chunk:
        ou[0:64]   += V_c.T @ E_c_blk0       (col-tile (0,0))
        ou[64:128] += V_c.T @ E_c_blk1       (col-tile (0,64))   } concurrent
  - denominators ride as 1-wide col-tiles into a separate PSUM bank
    (4 positions: chunk parity x block), collapsed to a [q,2] column
    per 128-q-chunk by one tiny matmul against a 0/1 selector.
  - Wo, per 128-q-chunk: row-tile pair (blk0 rows 0:64, blk1 rows 64:128
    against a partition-replicated [Wo_h; Wo_h]) -> two concurrent
    [128,512] matmuls.

K/V are projected only up to vlp = ceil(vl/128)*128; the boundary chunk
is masked after exp. Output partial is written f16 (relative precision
preserved under the host-side divide)."""

import math
import os
from contextlib import ExitStack

import ml_dtypes
import numpy as np

import concourse.bass as bass
import concourse.mybir as mybir
import concourse.tile as tile
from concourse import bacc
from concourse import bass_utils

F32 = mybir.dt.float32
F16 = mybir.dt.float16
EXP = mybir.ActivationFunctionType.Exp

N_CORES = 8

# Problem shape (hardcoded per harness contract).
B_, S_, D_, H_ = 2, 4096, 512, 8
HD_ = D_ // H_
QB = 512          # q-block width
QP = 2 * QB       # q-block-pair width
ND = D_ // 128    # contraction chunks for the projections


def _ceil_div(a, b):
    return (a + b - 1) // b


def _blocks(total, width):
    out = []
    off = 0
    while off < total:
        out.append((off, min(width, total - off)))
        off += width
    return out


def build_kernel(nc, cfg):
    """Emit the per-core kernel IR. cfg keys: S, D, HD, vlps (tuple per
    batch, each a multiple of 128), dt, repeat."""
    S, D, HD = cfg["S"], cfg["D"], cfg["HD"]
    mdt = {"bf16": mybir.dt.bfloat16, "f16": F16, "f32": F32,
           "f32r": mybir.dt.float32r}[cfg.get("dt", "f16")]
    vlps = cfg["vlps"]
    B = len(vlps)
    scale = 1.0 / math.sqrt(HD)
    nch = [v // 128 for v in vlps]            # k-chunks per batch
    chbase = [sum(nch[:b]) for b in range(B)]
    nch_tot = sum(nch)
    nqp = S // QP                             # q-block-pairs per batch

    # ---- DRAM I/O ----
    qT = nc.dram_tensor("qT", [B, D, S], mdt, kind="ExternalInput").ap()
    kTs = [nc.dram_tensor(f"kT{b}", [D, vlps[b]], mdt, kind="ExternalInput").ap()
           for b in range(B)]
    vTs = [nc.dram_tensor(f"vT{b}", [D, vlps[b]], mdt, kind="ExternalInput").ap()
           for b in range(B)]
    wq = nc.dram_tensor("wq", [D, HD], mdt, kind="ExternalInput").ap()
    wk = nc.dram_tensor("wk", [D, HD], mdt, kind="ExternalInput").ap()
    wv = nc.dram_tensor("wv", [D, HD], mdt, kind="ExternalInput").ap()
    wo2 = nc.dram_tensor("wo2", [2 * HD, D], mdt, kind="ExternalInput").ap()
    mask = nc.dram_tensor("mask", [128, nch_tot], F32, kind="ExternalInput").ap()
    out = nc.dram_tensor("out", [B, S, D], F16, kind="ExternalOutput").ap()
    # denominators: [128, b, qpair, qchunk, blk]
    den_out = nc.dram_tensor("den", [128, B, nqp, 4, 2], F32,
                             kind="ExternalOutput").ap()
    if cfg.get("dump"):
        qT_dump = nc.dram_tensor("qT_dump", [128, B, nqp, QB], F16,
                                 kind="ExternalOutput").ap()
        kT_dump = nc.dram_tensor("kT_dump", [128, B, max(vlps)], F16,
                                 kind="ExternalOutput").ap()
        vb_dump = nc.dram_tensor("vb_dump", [128, nch_tot, HD], F16,
                                 kind="ExternalOutput").ap()
        e_dump = nc.dram_tensor("e_dump", [128, 2, 2, QB], F16,
                                kind="ExternalOutput").ap()
        ou_dump = nc.dram_tensor("ou_dump", [128, QB], F16,
                                 kind="ExternalOutput").ap()
        dn_dump = nc.dram_tensor("dn_dump", [128, QB], F32,
                                 kind="ExternalOutput").ap()

    with tile.TileContext(nc) as tc, ExitStack() as ctx:
        consts = ctx.enter_context(tc.tile_pool(name="consts", bufs=1))
        xt = ctx.enter_context(tc.tile_pool(name="xt", bufs=2 * ND + 2))
        qkv = ctx.enter_context(tc.tile_pool(name="qkv", bufs=1))
        epool = ctx.enter_context(tc.tile_pool(name="e", bufs=4))
        ousb = ctx.enter_context(tc.tile_pool(name="ousb", bufs=2))
        densb = ctx.enter_context(tc.tile_pool(name="densb", bufs=2))
        stage = ctx.enter_context(tc.tile_pool(name="stage", bufs=2))
        dall = ctx.enter_context(tc.tile_pool(name="dall", bufs=1))
        # PSUM: ssc 2x2 banks + ou 1 + den 1 + mm 2  = 8 banks
        ps_sc = ctx.enter_context(tc.tile_pool(name="ps_sc", bufs=2, space="PSUM"))
        ps_ou = ctx.enter_context(tc.tile_pool(name="ps_ou", bufs=1, space="PSUM"))
        ps_den = ctx.enter_context(tc.tile_pool(name="ps_den", bufs=1, space="PSUM"))
        ps_mm = ctx.enter_context(tc.tile_pool(name="ps_mm", bufs=2, space="PSUM"))

        # ---- constants ----
        wq_sb = consts.tile([128, ND, HD], mdt)
        wk_sb = consts.tile([128, ND, HD], mdt)
        wv_sb = consts.tile([128, ND, HD], mdt)
        for w_sb, w_ap in ((wq_sb, wq), (wk_sb, wk), (wv_sb, wv)):
            nc.sync.dma_start(out=w_sb, in_=w_ap.rearrange("(c p) h -> p c h", p=128))
        wo2_sb = consts.tile([2 * HD, D], mdt)
        nc.sync.dma_start(out=wo2_sb, in_=wo2)
        mask_sb = consts.tile([128, nch_tot], F32)
        nc.sync.dma_start(out=mask_sb, in_=mask)
        # all-ones column: lhsT of the denominator 1-wide col-tiles
        ones_f32 = consts.tile([128, 1], F32)
        nc.vector.memset(ones_f32, 1.0)
        ones_sb = consts.tile([128, 1], mdt)
        nc.vector.tensor_copy(ones_sb, ones_f32)
        # selector for the denominator collapse: col blk sums rows
        # {32*(2*par+blk) : par in 0,1} (restricted to parities that occur)
        sel = consts.tile([128, B, 2], F32)
        nc.vector.memset(sel, 0.0)
        for b in range(B):
            for blk in range(2):
                for par in range(2):
                    if nch[b] > par:
                        r = 32 * (2 * par + blk)
                        nc.vector.memset(sel[r : r + 1, b, blk : blk + 1], 1.0)

        def emit():
            qT_sb = qkv.tile([128, B, nqp, QB], mdt)     # blk0 rows 0:64, blk1 64:128
            kT_sb = qkv.tile([128, B, max(vlps)], mdt)   # replicated both halves
            vbuf = qkv.tile([128, nch_tot, HD], mdt)
            den_all = dall.tile([128, B, nqp, 4, 2], F32)

            # ---------- projection steps (interleavable units) ----------
            # Q: project two 512-blocks at once; block 2p -> psum rows
            # 0:64 (col-tile (0,0)), block 2p+1 -> rows 64:128 ((0,64)).
            def q_step(b, p):
                def step():
                    tiles = []
                    for dc in range(ND):
                        t = xt.tile([128, QP], mdt, tag="xt")
                        nc.sync.dma_start(
                            out=t,
                            in_=qT[b, dc * 128 : (dc + 1) * 128,
                                   p * QP : (p + 1) * QP],
                        )
                        tiles.append(t)
                    ps = ps_mm.tile([128, QB], F32, name="ps", tag="mm")
                    for dc in range(ND):
                        nc.tensor.matmul(
                            ps[0:64, :], wq_sb[:, dc, :], tiles[dc][:, 0:QB],
                            start=(dc == 0), stop=(dc == ND - 1),
                            skip_group_check=True)
                        nc.tensor.matmul(
                            ps[64:128, :], wq_sb[:, dc, :], tiles[dc][:, QB:QP],
                            start=(dc == 0), stop=(dc == ND - 1),
                            skip_group_check=True)
                    nc.vector.tensor_copy(qT_sb[:, b, p, :], ps)
                return step

            # K: replicated on both halves (two col-tiles, same rhs).
            def k_step(b, soff, sw):
                def step():
                    tiles = []
                    for dc in range(ND):
                        t = xt.tile([128, QP], mdt, tag="xt")
                        nc.sync.dma_start(
                            out=t[:, :sw],
                            in_=kTs[b][dc * 128 : (dc + 1) * 128, soff : soff + sw])
                        tiles.append(t)
                    ps = ps_mm.tile([128, QB], F32, name="ps", tag="mm")
                    for dc in range(ND):
                        nc.tensor.matmul(
                            ps[0:64, :sw], wk_sb[:, dc, :], tiles[dc][:, :sw],
                            start=(dc == 0), stop=(dc == ND - 1),
                            skip_group_check=True)
                        nc.tensor.matmul(
                            ps[64:128, :sw], wk_sb[:, dc, :], tiles[dc][:, :sw],
                            start=(dc == 0), stop=(dc == ND - 1),
                            skip_group_check=True)
                    nc.vector.tensor_copy(kT_sb[:, b, soff : soff + sw],
                                          ps[:, :sw])
                return step

            # V: natural orientation per 128-chunk.
            def v_step(b, soff, sw):
                def step():
                    tiles = []
                    for dc in range(ND):
                        t = xt.tile([128, QP], mdt, tag="xt")
                        nc.sync.dma_start(
                            out=t[:, :sw],
                            in_=vTs[b][dc * 128 : (dc + 1) * 128, soff : soff + sw])
                        tiles.append(t)
                    nsub = sw // 128
                    ps = ps_mm.tile([128, 4, HD], F32, name="ps", tag="mm")
                    for sub in range(nsub):
                        for dc in range(ND):
                            nc.tensor.matmul(
                                ps[:, sub, :],
                                tiles[dc][:, sub * 128 : (sub + 1) * 128],
                                wv_sb[:, dc, :],
                                start=(dc == 0),
                                stop=(dc == ND - 1),
                                skip_group_check=True)
                    kc = chbase[b] + soff // 128
                    nc.vector.tensor_copy(vbuf[:, kc : kc + nsub, :],
                                          ps[:, :nsub, :])
                return step

            def proj_steps(b):
                """K/V/Q interleaved so low chunks become available first."""
                kv = _blocks(vlps[b], QB)
                steps = []
                for i in range(max(len(kv), nqp)):
                    if i < len(kv):
                        steps.append(k_step(b, kv[i][0], kv[i][1]))
                        steps.append(v_step(b, kv[i][0], kv[i][1]))
                    if i < nqp:
                        steps.append(q_step(b, i))
                return steps

            # ---------- attention for batch b ----------
            # Software-pipelined: for each chunk, the scores matmuls and exp
            # of chunk c are emitted BEFORE the consumers (outU/den matmuls)
            # of chunk c-1, so the PE never sits behind an ACT-dependent
            # instruction while ACT waits for PE-produced scores. The
            # per-qpair tail (Wo etc.) is deferred into the next qpair's
            # chunk loop for the same reason.
            tails = []          # deferred tail closures
            proj_q = []         # pending projection steps, pumped per chunk

            def flush_tails():
                while tails:
                    tails.pop(0)()

            def pump():
                if proj_q:
                    proj_q.pop(0)()

            def phase_b(b):
                nb = nch[b]
                vfrac = vlps[b] != cfg["vls"][b]   # boundary chunk needs mask

                def produce(p, c, ou_den):
                    qlo = qT_sb[0:64, b, p, :]
                    qhi = qT_sb[64:128, b, p, :]
                    kcol = slice(c * 128, (c + 1) * 128)
                    ssc = ps_sc.tile([128, 2, QB], F32)   # [blk, q]
                    if not cfg.get("skip_scores"):
                        nc.tensor.matmul(
                            ssc[:, 0, :], kT_sb[0:64, b, kcol], qlo,
                            start=True, stop=True, skip_group_check=True)
                        nc.tensor.matmul(
                            ssc[:, 1, :], kT_sb[64:128, b, kcol], qhi,
                            start=True, stop=True, skip_group_check=True)
                    e = epool.tile([128, 2, QB], mdt, tag="e")
                    if cfg.get("skip_exp"):
                        pass
                    elif cfg.get("no_exp"):
                        nc.vector.tensor_copy(e, ssc)
                    elif cfg.get("cheap_exp"):
                        nc.scalar.activation(e[:, :, 0:64], ssc[:, :, 0:64],
                                             EXP, bias=0.0, scale=scale)
                    else:
                        nc.scalar.activation(e, ssc, EXP, bias=0.0, scale=scale)
                    return e

                def consume(p, c, e, epair, ou_den):
                    ou, den = ou_den
                    if vfrac and c == nb - 1 and not cfg.get("skip_exp"):
                        mc = chbase[b] + c
                        nc.vector.tensor_scalar_mul(
                            e, e, mask_sb[:, mc : mc + 1])
                    if cfg.get("dump") and b == 0 and p == 0 and c < 2:
                        nc.sync.dma_start(out=e_dump[:, c, :, :], in_=e)
                    if cfg.get("skip_outu"):
                        return
                    vb_c = vbuf[:, chbase[b] + c, :]
                    nc.tensor.matmul(
                        ou[0:64, :], vb_c, e[:, 0, :],
                        start=(c == 0), stop=False, skip_group_check=True)
                    nc.tensor.matmul(
                        ou[64:128, :], vb_c, e[:, 1, :],
                        start=(c == 0), stop=(c == nb - 1),
                        skip_group_check=True)
                    # denominator 1-wide col-tiles: issue in groups of 4
                    # (chunk pair x block) so all four run concurrently in
                    # distinct 32-col strips of the PE array.
                    epair[c % 2] = e
                    if c % 2 == 1 or c == nb - 1:
                        for i in range(2 if c % 2 == 1 else 1):
                            cc = c - (1 if c % 2 == 1 else 0) + i
                            for blk in range(2):
                                r = 32 * (2 * (cc % 2) + blk)
                                nc.tensor.matmul(
                                    den[r : r + 1, :], ones_sb,
                                    epair[cc % 2][:, blk, :],
                                    start=(cc < 2), stop=(cc >= nb - 2),
                                    skip_group_check=True,
                                    tile_position=(0, r))

                def make_tail(p, ou_den):
                    ou, den = ou_den

                    def tail():
                        ou_sb = ousb.tile([128, QB], mdt)
                        den_sb = densb.tile([128, QB], F32)
                        if cfg.get("skip_outu"):
                            nc.vector.memset(ou_sb, 1.0)
                            nc.vector.memset(den_sb, 1.0)
                        else:
                            nc.vector.tensor_copy(ou_sb, ou)
                            nc.vector.tensor_copy(den_sb, den)
                        if cfg.get("dump") and b == 0 and p == 0:
                            nc.sync.dma_start(out=ou_dump, in_=ou_sb)
                            nc.sync.dma_start(out=dn_dump, in_=den_sb)
                        dps = ps_mm.tile([128, 4, 2], F32, tag="mm")
                        for qc in range(4):
                            nc.tensor.matmul(
                                dps[:, qc, :],
                                den_sb[:, qc * 128 : (qc + 1) * 128],
                                sel[:, b, :], start=True, stop=True,
                                skip_group_check=True)
                        nc.vector.tensor_copy(den_all[:, b, p, :, :], dps)
                        st = stage.tile([128, 8, D], F16)
                        for qc in range(4):
                            qcol = slice(qc * 128, (qc + 1) * 128)
                            wps0 = ps_mm.tile([128, D], F32, tag="mm")
                            wps1 = ps_mm.tile([128, D], F32, tag="mm")
                            nc.tensor.matmul(
                                wps0, ou_sb[0:64, qcol], wo2_sb[0:64, :],
                                start=True, stop=True, skip_group_check=True)
                            nc.tensor.matmul(
                                wps1, ou_sb[64:128, qcol], wo2_sb[64:128, :],
                                start=True, stop=True, skip_group_check=True)
                            nc.vector.tensor_copy(st[:, qc, :], wps0)
                            nc.vector.tensor_copy(st[:, 4 + qc, :], wps1)
                        if not cfg.get("no_out_dma"):
                            nc.sync.dma_start(
                                out=out[b, p * QP : (p + 1) * QP, :].rearrange(
                                    "(q pp) n -> pp q n", pp=128),
                                in_=st)
                    return tail

                for p in range(nqp):
                    ou_den = (ps_ou.tile([128, QB], F32, name="ou", tag="ou"),
                              ps_den.tile([128, QB], F32, name="den", tag="den"))
                    epair = [None, None]
                    pend = None
                    for c in range(nb):
                        e = produce(p, c, ou_den)
                        if c == 1:
                            flush_tails()
                        pump()
                        if pend is not None:
                            consume(p, c - 1, pend, epair, ou_den)
                        pend = e
                    if nb == 1:
                        flush_tails()
                    consume(p, nb - 1, pend, epair, ou_den)
                    tails.append(make_tail(p, ou_den))

            # Head: emit just enough of batch 0's projections (K/V block 0,
            # Q pair 0) for attention to start; the rest of batch 0's and all
            # of batch 1's projections are pumped into the attention loop,
            # where the PE has slack while ACT streams exp.
            for b in range(B):
                proj_q.extend(proj_steps(b))
            for _ in range(3):
                pump()
            for b in range(B):
                if b > 0:
                    # batch b's projections must all be emitted before its
                    # attention begins (normally drained long before this)
                    while proj_q:
                        pump()
                phase_b(b)
            flush_tails()
            nc.sync.dma_start(out=den_out, in_=den_all)
            if cfg.get("dump"):
                nc.sync.dma_start(out=qT_dump, in_=qT_sb)
                nc.sync.dma_start(out=kT_dump, in_=kT_sb)
                nc.sync.dma_start(out=vb_dump, in_=vbuf)

        for _ in range(cfg.get("repeat", 1)):
            emit()

    nc.compile()
    return nc


def prepare_in_maps(queries, keys, values, vls, Wq, Wk, Wv, Wo, vlps,
                    np_dt=np.float16):
    """Host-side layout prep: transposes, trims, per-core weight slices, mask."""
    HD = HD_
    qT = np.ascontiguousarray(
        queries.transpose(0, 2, 1).astype(np_dt))          # [B, D, S]
    kT = [np.ascontiguousarray(keys[b].T[:, : vlps[b]].astype(np_dt))
          for b in range(B_)]
    vT = [np.ascontiguousarray(values[b].T[:, : vlps[b]].astype(np_dt))
          for b in range(B_)]
    nch = [v // 128 for v in vlps]
    mask_np = np.zeros((128, sum(nch)), dtype=np.float32)
    cb = 0
    for b in range(B_):
        idx = np.arange(vlps[b]).reshape(nch[b], 128).T    # [128, nch]
        mask_np[:, cb : cb + nch[b]] = (idx < vls[b]).astype(np.float32)
        cb += nch[b]

    in_maps = []
    for c in range(N_CORES):
        h0 = c * HD
        woh = Wo[h0 : h0 + HD, :].astype(np_dt)
        m = {
            "qT": qT,
            "wq": np.ascontiguousarray(Wq[:, h0 : h0 + HD].astype(np_dt)),
            "wk": np.ascontiguousarray(Wk[:, h0 : h0 + HD].astype(np_dt)),
            "wv": np.ascontiguousarray(Wv[:, h0 : h0 + HD].astype(np_dt)),
            "wo2": np.ascontiguousarray(np.concatenate([woh, woh], axis=0)),
            "mask": mask_np,
        }
        for b in range(B_):
            m[f"kT{b}"] = kT[b]
            m[f"vT{b}"] = vT[b]
        in_maps.append(m)
    return in_maps


_NC_CACHE = {}

DEFAULT_DT = os.environ.get("KERNEL_DT", "f16")


def _get_nc(cfg_key):
    if cfg_key not in _NC_CACHE:
        S, D, HD, vlps, vls, dt = cfg_key
        nc = bacc.Bacc(
            "TRN2",
            target_bir_lowering=False,
            debug=False,
            enable_asserts=False,
            num_devices=N_CORES,
        )
        build_kernel(nc, {"S": S, "D": D, "HD": HD, "vlps": vlps,
                          "vls": vls, "dt": dt})
        _NC_CACHE[cfg_key] = nc
    return _NC_CACHE[cfg_key]


LAST_RESULT = None
LAST_IN_MAPS = None


def kernel(queries, keys, values, valid_lens, Wq, Wk, Wv, Wo, _trace=False):
    global LAST_RESULT, LAST_IN_MAPS
    queries = np.ascontiguousarray(np.asarray(queries, dtype=np.float32))
    keys = np.ascontiguousarray(np.asarray(keys, dtype=np.float32))
    values = np.ascontiguousarray(np.asarray(values, dtype=np.float32))
    Wq = np.ascontiguousarray(np.asarray(Wq, dtype=np.float32))
    Wk = np.ascontiguousarray(np.asarray(Wk, dtype=np.float32))
    Wv = np.ascontiguousarray(np.asarray(Wv, dtype=np.float32))
    Wo = np.ascontiguousarray(np.asarray(Wo, dtype=np.float32))
    vls = tuple(int(v) for v in np.asarray(valid_lens).reshape(-1))

    Bq, S, D = queries.shape
    assert (Bq, S, D) == (B_, S_, D_), (Bq, S, D)
    HD = HD_
    vlps = tuple(min(S, _ceil_div(max(v, 1), 128) * 128) for v in vls)

    dt = DEFAULT_DT
    nc = _get_nc((S, D, HD, vlps, vls, dt))
    np_dt = {"bf16": ml_dtypes.bfloat16, "f16": np.float16}.get(dt, np.float32)
    in_maps = prepare_in_maps(
        queries, keys, values, vls, Wq, Wk, Wv, Wo, vlps, np_dt=np_dt)
    LAST_IN_MAPS = in_maps
    LAST_RESULT = bass_utils.run_bass_kernel_spmd(
        nc, in_maps, core_ids=list(range(N_CORES)), trace=_trace)
    acc = np.zeros((B_, S, D), dtype=np.float32)
    for r in LAST_RESULT.results:
        # den layout: [128, B, qpair, qchunk, blk] -> [B, S]
        den = np.asarray(r["den"], dtype=np.float32)
        den = den.transpose(1, 2, 4, 3, 0).reshape(B_, S)
        acc += np.asarray(r["out"], dtype=np.float32) / den[:, :, None]
    return acc


# revision 28
# speedup vs baseline: 11.4034x; 11.4034x over previous
"""Multi-head attention (B=2, S=4096, D=512, H=8) on 8 trn2 NeuronCores.

Sharding: head-parallel. Core i computes head i for BOTH batches (work per
head is proportional to that batch's valid_len, so pairing each head with
both batches balances the skewed valid_lens across cores). Each core
applies its row-slice of Wo on device and returns a full-shape
UNNORMALIZED partial plus per-query softmax denominators; the host
divides by the denominator and sums the 8 partials (the tensor-parallel
all-reduce, done in the gather step).

Device dataflow per core (matmuls in fp16; PSUM accumulates fp32).
Queries are processed in PAIRS of 512-wide blocks: block 2p lives on SBUF
partitions 0:64 (its 64 head dims), block 2p+1 on partitions 64:128.
K^T is replicated on both partition halves. This lets every stage use
PE-array tiling (tile_position) so two matmuls run concurrently:

  - scores, per (q-block-pair, k-chunk):
        ssc[k,q]  blk0 = kT_lo.T @ qT_lo     (row-tile (0,0))
        ssc[k,q]  blk1 = kT_hi.T @ qT_hi     (row-tile (64,0))   } concurrent
  - exp on ACT in 2-chunk batches (N=2048) to amortize the ~350cy
    per-instruction overhead; boundary chunk masked multiplicatively
    afterwards on DVE (per-partition 0/1 scalar).
  - outU accumulation, per k-chunk:
        ou[0:64]   += V_c.T @ E_c_blk0       (col-tile (0,0))
        ou[64:128] += V_c.T @ E_c_blk1       (col-tile (0,64))   } concurrent
  - denominators ride as 1-wide col-tiles into a separate PSUM bank
    (4 positions: chunk parity x block), collapsed to a [q,2] column
    per 128-q-chunk by one tiny matmul against a 0/1 selector.
  - Wo, per 128-q-chunk: row-tile pair (blk0 rows 0:64, blk1 rows 64:128
    against a partition-replicated [Wo_h; Wo_h]) -> two concurrent
    [128,512] matmuls.

K/V are projected only up to vlp = ceil(vl/128)*128; the boundary chunk
is masked after exp. Output partial is written f16 (relative precision
preserved under the host-side divide)."""

import math
import os
from contextlib import ExitStack

import ml_dtypes
import numpy as np

import concourse.bass as bass
import concourse.mybir as mybir
import concourse.tile as tile
from concourse import bacc
from concourse import bass_utils

F32 = mybir.dt.float32
F16 = mybir.dt.float16
EXP = mybir.ActivationFunctionType.Exp

N_CORES = 8

# Problem shape (hardcoded per harness contract).
B_, S_, D_, H_ = 2, 4096, 512, 8
HD_ = D_ // H_
QB = 512          # q-block width
QP = 2 * QB       # q-block-pair width
ND = D_ // 128    # contraction chunks for the projections


def _ceil_div(a, b):
    return (a + b - 1) // b


def _blocks(total, width):
    out = []
    off = 0
    while off < total:
        out.append((off, min(width, total - off)))
        off += width
    return out


def build_kernel(nc, cfg):
    """Emit the per-core kernel IR. cfg keys: S, D, HD, vlps (tuple per
    batch, each a multiple of 128), dt, repeat."""
    S, D, HD = cfg["S"], cfg["D"], cfg["HD"]
    mdt = {"bf16": mybir.dt.bfloat16, "f16": F16, "f32": F32,
           "f32r": mybir.dt.float32r}[cfg.get("dt", "f16")]
    vlps = cfg["vlps"]
    B = len(vlps)
    scale = 1.0 / math.sqrt(HD)
    nch = [v // 128 for v in vlps]            # k-chunks per batch
    chbase = [sum(nch[:b]) for b in range(B)]
    nch_tot = sum(nch)
    nqp = S // QP                             # q-block-pairs per batch

    # ---- DRAM I/O ----
    qT = nc.dram_tensor("qT", [B, D, S], mdt, kind="ExternalInput").ap()
    kTs = [nc.dram_tensor(f"kT{b}", [D, vlps[b]], mdt, kind="ExternalInput").ap()
           for b in range(B)]
    vTs = [nc.dram_tensor(f"vT{b}", [D, vlps[b]], mdt, kind="ExternalInput").ap()
           for b in range(B)]
    wq = nc.dram_tensor("wq", [D, HD], mdt, kind="ExternalInput").ap()
    wk = nc.dram_tensor("wk", [D, HD], mdt, kind="ExternalInput").ap()
    wv = nc.dram_tensor("wv", [D, HD], mdt, kind="ExternalInput").ap()
    wo2 = nc.dram_tensor("wo2", [2 * HD, D], mdt, kind="ExternalInput").ap()
    mask = nc.dram_tensor("mask", [128, nch_tot], F32, kind="ExternalInput").ap()
    out = nc.dram_tensor("out", [B, S, D], F16, kind="ExternalOutput").ap()
    # denominators: [128, b, qpair, qchunk, blk]
    den_out = nc.dram_tensor("den", [128, B, nqp, 4, 2], F32,
                             kind="ExternalOutput").ap()
    if cfg.get("dump"):
        qT_dump = nc.dram_tensor("qT_dump", [128, B, nqp, QB], F16,
                                 kind="ExternalOutput").ap()
        kT_dump = nc.dram_tensor("kT_dump", [128, B, max(vlps)], F16,
                                 kind="ExternalOutput").ap()
        vb_dump = nc.dram_tensor("vb_dump", [128, nch_tot, HD], F16,
                                 kind="ExternalOutput").ap()
        e_dump = nc.dram_tensor("e_dump", [128, 2, 2, QB], F16,
                                kind="ExternalOutput").ap()
        ou_dump = nc.dram_tensor("ou_dump", [128, QB], F16,
                                 kind="ExternalOutput").ap()
        dn_dump = nc.dram_tensor("dn_dump", [128, QB], F32,
                                 kind="ExternalOutput").ap()

    with tile.TileContext(nc) as tc, ExitStack() as ctx:
        consts = ctx.enter_context(tc.tile_pool(name="consts", bufs=1))
        xt = ctx.enter_context(tc.tile_pool(name="xt", bufs=2 * ND + 2))
        qkv = ctx.enter_context(tc.tile_pool(name="qkv", bufs=1))
        epool = ctx.enter_context(tc.tile_pool(name="e", bufs=6))
        ousb = ctx.enter_context(tc.tile_pool(name="ousb", bufs=3))
        densb = ctx.enter_context(tc.tile_pool(name="densb", bufs=3))
        stage = ctx.enter_context(tc.tile_pool(name="stage", bufs=3))
        dall = ctx.enter_context(tc.tile_pool(name="dall", bufs=1))
        # PSUM: ssc 2x2 banks + ou 1 + den 1 + mm 2  = 8 banks
        ps_sc = ctx.enter_context(tc.tile_pool(name="ps_sc", bufs=2, space="PSUM"))
        ps_ou = ctx.enter_context(tc.tile_pool(name="ps_ou", bufs=1, space="PSUM"))
        ps_den = ctx.enter_context(tc.tile_pool(name="ps_den", bufs=1, space="PSUM"))
        ps_mm = ctx.enter_context(tc.tile_pool(name="ps_mm", bufs=2, space="PSUM"))

        # ---- constants ----
        wq_sb = consts.tile([128, ND, HD], mdt)
        wk_sb = consts.tile([128, ND, HD], mdt)
        wv_sb = consts.tile([128, ND, HD], mdt)
        for w_sb, w_ap in ((wq_sb, wq), (wk_sb, wk), (wv_sb, wv)):
            nc.sync.dma_start(out=w_sb, in_=w_ap.rearrange("(c p) h -> p c h", p=128))
        wo2_sb = consts.tile([2 * HD, D], mdt)
        nc.sync.dma_start(out=wo2_sb, in_=wo2)
        mask_sb = consts.tile([128, nch_tot], F32)
        nc.sync.dma_start(out=mask_sb, in_=mask)
        # all-ones column: lhsT of the denominator 1-wide col-tiles
        ones_f32 = consts.tile([128, 1], F32)
        nc.vector.memset(ones_f32, 1.0)
        ones_sb = consts.tile([128, 1], mdt)
        nc.vector.tensor_copy(ones_sb, ones_f32)
        # selector for the denominator collapse: col blk sums rows
        # {32*(2*par+blk) : par in 0,1} (restricted to parities that occur)
        sel = consts.tile([128, B, 2], F32)
        nc.vector.memset(sel, 0.0)
        for b in range(B):
            for blk in range(2):
                for par in range(2):
                    if nch[b] > par:
                        r = 32 * (2 * par + blk)
                        nc.vector.memset(sel[r : r + 1, b, blk : blk + 1], 1.0)

        def emit():
            qT_sb = qkv.tile([128, B, nqp, QB], mdt)     # blk0 rows 0:64, blk1 64:128
            kT_sb = qkv.tile([128, B, max(vlps)], mdt)   # replicated both halves
            vbuf = qkv.tile([128, nch_tot, HD], mdt)
            den_all = dall.tile([128, B, nqp, 4, 2], F32)

            # ---------- projection steps (interleavable units) ----------
            # Q: project two 512-blocks at once; block 2p -> psum rows
            # 0:64 (col-tile (0,0)), block 2p+1 -> rows 64:128 ((0,64)).
            def q_step(b, p):
                def step():
                    tiles = []
                    for dc in range(ND):
                        t = xt.tile([128, QP], mdt, tag="xt")
                        nc.sync.dma_start(
                            out=t,
                            in_=qT[b, dc * 128 : (dc + 1) * 128,
                                   p * QP : (p + 1) * QP],
                        )
                        tiles.append(t)
                    ps = ps_mm.tile([128, QB], F32, name="ps", tag="mm")
                    for dc in range(ND):
                        nc.tensor.matmul(
                            ps[0:64, :], wq_sb[:, dc, :], tiles[dc][:, 0:QB],
                            start=(dc == 0), stop=(dc == ND - 1),
                            skip_group_check=True)
                        nc.tensor.matmul(
                            ps[64:128, :], wq_sb[:, dc, :], tiles[dc][:, QB:QP],
                            start=(dc == 0), stop=(dc == ND - 1),
                            skip_group_check=True)
                    nc.vector.tensor_copy(qT_sb[:, b, p, :], ps)
                return step

            # K: replicated on both halves (two col-tiles, same rhs).
            def k_step(b, soff, sw):
                def step():
                    tiles = []
                    for dc in range(ND):
                        t = xt.tile([128, QP], mdt, tag="xt")
                        nc.sync.dma_start(
                            out=t[:, :sw],
                            in_=kTs[b][dc * 128 : (dc + 1) * 128, soff : soff + sw])
                        tiles.append(t)
                    ps = ps_mm.tile([128, QB], F32, name="ps", tag="mm")
                    for dc in range(ND):
                        nc.tensor.matmul(
                            ps[0:64, :sw], wk_sb[:, dc, :], tiles[dc][:, :sw],
                            start=(dc == 0), stop=(dc == ND - 1),
                            skip_group_check=True)
                        nc.tensor.matmul(
                            ps[64:128, :sw], wk_sb[:, dc, :], tiles[dc][:, :sw],
                            start=(dc == 0), stop=(dc == ND - 1),
                            skip_group_check=True)
                    nc.vector.tensor_copy(kT_sb[:, b, soff : soff + sw],
                                          ps[:, :sw])
                return step

            # V: natural orientation per 128-chunk.
            def v_step(b, soff, sw):
                def step():
                    tiles = []
                    for dc in range(ND):
                        t = xt.tile([128, QP], mdt, tag="xt")
                        nc.sync.dma_start(
                            out=t[:, :sw],
                            in_=vTs[b][dc * 128 : (dc + 1) * 128, soff : soff + sw])
                        tiles.append(t)
                    nsub = sw // 128
                    ps = ps_mm.tile([128, 4, HD], F32, name="ps", tag="mm")
                    for sub in range(nsub):
                        for dc in range(ND):
                            nc.tensor.matmul(
                                ps[:, sub, :],
                                tiles[dc][:, sub * 128 : (sub + 1) * 128],
                                wv_sb[:, dc, :],
                                start=(dc == 0),
                                stop=(dc == ND - 1),
                                skip_group_check=True)
                    kc = chbase[b] + soff // 128
                    nc.vector.tensor_copy(vbuf[:, kc : kc + nsub, :],
                                          ps[:, :nsub, :])
                return step

            def proj_steps(b):
                """K/V/Q interleaved so low chunks become available first."""
                kv = _blocks(vlps[b], QB)
                steps = []
                for i in range(max(len(kv), nqp)):
                    if i < len(kv):
                        steps.append(k_step(b, kv[i][0], kv[i][1]))
                        steps.append(v_step(b, kv[i][0], kv[i][1]))
                    if i < nqp:
                        steps.append(q_step(b, i))
                return steps

            # ---------- attention for batch b ----------
            # Software-pipelined: for each chunk, the scores matmuls and exp
            # of chunk c are emitted BEFORE the consumers (outU/den matmuls)
            # of chunk c-1, so the PE never sits behind an ACT-dependent
            # instruction while ACT waits for PE-produced scores. The
            # per-qpair tail (Wo etc.) is deferred into the next qpair's
            # chunk loop for the same reason.
            tails = []          # deferred tail closures
            proj_q = []         # pending projection steps, pumped per chunk

            def flush_tails():
                while tails:
                    tails.pop(0)()

            def pump():
                if proj_q:
                    proj_q.pop(0)()

            def phase_b(b):
                nb = nch[b]
                vfrac = vlps[b] != cfg["vls"][b]   # boundary chunk needs mask

                def produce(p, c, ou_den):
                    qlo = qT_sb[0:64, b, p, :]
                    qhi = qT_sb[64:128, b, p, :]
                    kcol = slice(c * 128, (c + 1) * 128)
                    ssc = ps_sc.tile([128, 2, QB], F32)   # [blk, q]
                    if not cfg.get("skip_scores"):
                        nc.tensor.matmul(
                            ssc[:, 0, :], kT_sb[0:64, b, kcol], qlo,
                            start=True, stop=True, skip_group_check=True)
                        nc.tensor.matmul(
                            ssc[:, 1, :], kT_sb[64:128, b, kcol], qhi,
                            start=True, stop=True, skip_group_check=True)
                    e = epool.tile([128, 2, QB], mdt, tag="e")
                    if cfg.get("skip_exp"):
                        pass
                    elif cfg.get("no_exp"):
                        nc.vector.tensor_copy(e, ssc)
                    elif cfg.get("cheap_exp"):
                        nc.scalar.activation(e[:, :, 0:64], ssc[:, :, 0:64],
                                             EXP, bias=0.0, scale=scale)
                    else:
                        nc.scalar.activation(e, ssc, EXP, bias=0.0, scale=scale)
                    return e

                def consume(p, c, e, epair, ou_den):
                    ou, den = ou_den
                    if vfrac and c == nb - 1 and not cfg.get("skip_exp"):
                        mc = chbase[b] + c
                        nc.vector.tensor_scalar_mul(
                            e, e, mask_sb[:, mc : mc + 1])
                    if cfg.get("dump") and b == 0 and p == 0 and c < 2:
                        nc.sync.dma_start(out=e_dump[:, c, :, :], in_=e)
                    if cfg.get("skip_outu"):
                        return
                    vb_c = vbuf[:, chbase[b] + c, :]
                    nc.tensor.matmul(
                        ou[0:64, :], vb_c, e[:, 0, :],
                        start=(c == 0), stop=False, skip_group_check=True)
                    nc.tensor.matmul(
                        ou[64:128, :], vb_c, e[:, 1, :],
                        start=(c == 0), stop=(c == nb - 1),
                        skip_group_check=True)
                    # denominator 1-wide col-tiles: issue in groups of 4
                    # (chunk pair x block) so all four run concurrently in
                    # distinct 32-col strips of the PE array.
                    epair[c % 2] = e
                    if c % 2 == 1 or c == nb - 1:
                        for i in range(2 if c % 2 == 1 else 1):
                            cc = c - (1 if c % 2 == 1 else 0) + i
                            for blk in range(2):
                                r = 32 * (2 * (cc % 2) + blk)
                                nc.tensor.matmul(
                                    den[r : r + 1, :], ones_sb,
                                    epair[cc % 2][:, blk, :],
                                    start=(cc < 2), stop=(cc >= nb - 2),
                                    skip_group_check=True,
                                    tile_position=(0, r))

                def make_tail(p, ou_den):
                    ou, den = ou_den

                    def tail():
                        ou_sb = ousb.tile([128, QB], mdt)
                        den_sb = densb.tile([128, QB], F32)
                        if cfg.get("skip_outu"):
                            nc.vector.memset(ou_sb, 1.0)
                            nc.vector.memset(den_sb, 1.0)
                        else:
                            nc.vector.tensor_copy(ou_sb, ou)
                            nc.vector.tensor_copy(den_sb, den)
                        if cfg.get("dump") and b == 0 and p == 0:
                            nc.sync.dma_start(out=ou_dump, in_=ou_sb)
                            nc.sync.dma_start(out=dn_dump, in_=den_sb)
                        dps = ps_mm.tile([128, 4, 2], F32, tag="mm")
                        for qc in range(4):
                            nc.tensor.matmul(
                                dps[:, qc, :],
                                den_sb[:, qc * 128 : (qc + 1) * 128],
                                sel[:, b, :], start=True, stop=True,
                                skip_group_check=True)
                        nc.vector.tensor_copy(den_all[:, b, p, :, :], dps)
                        st = stage.tile([128, 8, D], F16)
                        for qc in range(4):
                            qcol = slice(qc * 128, (qc + 1) * 128)
                            wps0 = ps_mm.tile([128, D], F32, tag="mm")
                            wps1 = ps_mm.tile([128, D], F32, tag="mm")
                            nc.tensor.matmul(
                                wps0, ou_sb[0:64, qcol], wo2_sb[0:64, :],
                                start=True, stop=True, skip_group_check=True)
                            nc.tensor.matmul(
                                wps1, ou_sb[64:128, qcol], wo2_sb[64:128, :],
                                start=True, stop=True, skip_group_check=True)
                            nc.vector.tensor_copy(st[:, qc, :], wps0)
                            if b == B - 1:
                                # during the short batch's attention ACT has
                                # slack; split the PSUM evacuation across
                                # both engines
                                nc.scalar.copy(st[:, 4 + qc, :], wps1)
                            else:
                                nc.vector.tensor_copy(st[:, 4 + qc, :], wps1)
                        if not cfg.get("no_out_dma"):
                            nc.sync.dma_start(
                                out=out[b, p * QP : (p + 1) * QP, :].rearrange(
                                    "(q pp) n -> pp q n", pp=128),
                                in_=st)
                    return tail

                for p in range(nqp):
                    ou_den = (ps_ou.tile([128, QB], F32, name="ou", tag="ou"),
                              ps_den.tile([128, QB], F32, name="den", tag="den"))
                    epair = [None, None]
                    pend = None
                    for c in range(nb):
                        e = produce(p, c, ou_den)
                        if c == 1:
                            flush_tails()
                        pump()
                        if pend is not None:
                            consume(p, c - 1, pend, epair, ou_den)
                        pend = e
                    if nb == 1:
                        flush_tails()
                    consume(p, nb - 1, pend, epair, ou_den)
                    tails.append(make_tail(p, ou_den))

            # Head: emit just enough of batch 0's projections (K/V block 0,
            # Q pair 0) for attention to start; the rest of batch 0's and all
            # of batch 1's projections are pumped into the attention loop,
            # where the PE has slack while ACT streams exp.
            for b in range(B):
                proj_q.extend(proj_steps(b))
            for _ in range(3):
                pump()
            for b in range(B):
                if b > 0:
                    # batch b's projections must all be emitted before its
                    # attention begins (normally drained long before this)
                    while proj_q:
                        pump()
                phase_b(b)
            flush_tails()
            nc.sync.dma_start(out=den_out, in_=den_all)
            if cfg.get("dump"):
                nc.sync.dma_start(out=qT_dump, in_=qT_sb)
                nc.sync.dma_start(out=kT_dump, in_=kT_sb)
                nc.sync.dma_start(out=vb_dump, in_=vbuf)

        for _ in range(cfg.get("repeat", 1)):
            emit()

    nc.compile()
    return nc


def prepare_in_maps(queries, keys, values, vls, Wq, Wk, Wv, Wo, vlps,
                    np_dt=np.float16):
    """Host-side layout prep: transposes, trims, per-core weight slices, mask."""
    HD = HD_
    qT = np.ascontiguousarray(
        queries.transpose(0, 2, 1).astype(np_dt))          # [B, D, S]
    kT = [np.ascontiguousarray(keys[b].T[:, : vlps[b]].astype(np_dt))
          for b in range(B_)]
    vT = [np.ascontiguousarray(values[b].T[:, : vlps[b]].astype(np_dt))
          for b in range(B_)]
    nch = [v // 128 for v in vlps]
    mask_np = np.zeros((128, sum(nch)), dtype=np.float32)
    cb = 0
    for b in range(B_):
        idx = np.arange(vlps[b]).reshape(nch[b], 128).T    # [128, nch]
        mask_np[:, cb : cb + nch[b]] = (idx < vls[b]).astype(np.float32)
        cb += nch[b]

    in_maps = []
    for c in range(N_CORES):
        h0 = c * HD
        woh = Wo[h0 : h0 + HD, :].astype(np_dt)
        m = {
            "qT": qT,
            "wq": np.ascontiguousarray(Wq[:, h0 : h0 + HD].astype(np_dt)),
            "wk": np.ascontiguousarray(Wk[:, h0 : h0 + HD].astype(np_dt)),
            "wv": np.ascontiguousarray(Wv[:, h0 : h0 + HD].astype(np_dt)),
            "wo2": np.ascontiguousarray(np.concatenate([woh, woh], axis=0)),
            "mask": mask_np,
        }
        for b in range(B_):
            m[f"kT{b}"] = kT[b]
            m[f"vT{b}"] = vT[b]
        in_maps.append(m)
    return in_maps


_NC_CACHE = {}

DEFAULT_DT = os.environ.get("KERNEL_DT", "f16")


def _get_nc(cfg_key):
    if cfg_key not in _NC_CACHE:
        S, D, HD, vlps, vls, dt = cfg_key
        nc = bacc.Bacc(
            "TRN2",
            target_bir_lowering=False,
            debug=False,
            enable_asserts=False,
            num_devices=N_CORES,
        )
        build_kernel(nc, {"S": S, "D": D, "HD": HD, "vlps": vlps,
                          "vls": vls, "dt": dt})
        _NC_CACHE[cfg_key] = nc
    return _NC_CACHE[cfg_key]


LAST_RESULT = None
LAST_IN_MAPS = None


def kernel(queries, keys, values, valid_lens, Wq, Wk, Wv, Wo, _trace=False):
    global LAST_RESULT, LAST_IN_MAPS
    queries = np.ascontiguousarray(np.asarray(queries, dtype=np.float32))
    keys = np.ascontiguousarray(np.asarray(keys, dtype=np.float32))
    values = np.ascontiguousarray(np.asarray(values, dtype=np.float32))
    Wq = np.ascontiguousarray(np.asarray(Wq, dtype=np.float32))
    Wk = np.ascontiguousarray(np.asarray(Wk, dtype=np.float32))
    Wv = np.ascontiguousarray(np.asarray(Wv, dtype=np.float32))
    Wo = np.ascontiguousarray(np.asarray(Wo, dtype=np.float32))
    vls = tuple(int(v) for v in np.asarray(valid_lens).reshape(-1))

    Bq, S, D = queries.shape
    assert (Bq, S, D) == (B_, S_, D_), (Bq, S, D)
    HD = HD_
    vlps = tuple(min(S, _ceil_div(max(v, 1), 128) * 128) for v in vls)

    dt = DEFAULT_DT
    nc = _get_nc((S, D, HD, vlps, vls, dt))
    np_dt = {"bf16": ml_dtypes.bfloat16, "f16": np.float16}.get(dt, np.float32)
    in_maps = prepare_in_maps(
        queries, keys, values, vls, Wq, Wk, Wv, Wo, vlps, np_dt=np_dt)
    LAST_IN_MAPS = in_maps
    LAST_RESULT = bass_utils.run_bass_kernel_spmd(
        nc, in_maps, core_ids=list(range(N_CORES)), trace=_trace)
    acc = np.zeros((B_, S, D), dtype=np.float32)
    for r in LAST_RESULT.results:
        # den layout: [128, B, qpair, qchunk, blk] -> [B, S]
        den = np.asarray(r["den"], dtype=np.float32)
        den = den.transpose(1, 2, 4, 3, 0).reshape(B_, S)
        acc += np.asarray(r["out"], dtype=np.float32) / den[:, :, None]
    return acc
